# revision 32
# baseline (speedup 1.0000x reference)
"""Trainium2 Bass kernel for the DVT sparse-attention module.

Strategy (8 NeuronCores, data-parallel):
  core = 2*s + h handles sample s (of 4) and half h (of 2) of its
  foreground queries.  The attention is *sparse*: queries are foreground
  positions (maxpool8(mask) > 0), keys are background positions, so each
  per-sample attention is only ~2048 x ~2048 instead of 4096 x 4096.
  The host compacts (gathers) fg/bg columns, the device runs
    feats = K_w @ [x; dvt] + K_b            (1x1 conv, contract 1024)
    Kn,Qn = column-normalized feats          (cosine prep)
    e     = exp(20 * Kn^T Qn)                (scores, [k, q] layout)
    denom = ones^T e ;  att = e / denom
    fore  = V e / denom ;  fused = fuse_w @ [fore; x_fg]
    vis   = sum_q att
  and the host scatters fused back into x (background positions pass
  through x unchanged) and assembles attmask from vis.

Everything runs in float32r on the PE (1 cycle/row for N>=256); exp /
normalization are fp32 with a Newton-refined reciprocal.
"""

import sys

sys.path.insert(0, "/opt/trn_rl_repo")

import numpy as np
import ml_dtypes  # noqa: F401  (bf16 numpy support)

import concourse.bass as bass
import concourse.mybir as mybir
from concourse.tile import TileContext
from concourse.bass_utils import run_bass_kernel_spmd

F32 = mybir.dt.float32
F32R = mybir.dt.float32r
BF16 = mybir.dt.bfloat16
AF = mybir.ActivationFunctionType
ALU = mybir.AluOpType

# capacities (inputs are deterministic; actual counts are ~2061 bg / ~1042 fg-half)
NB = 2176          # background-key capacity   (17 tiles of 128)
KT = NB // 128
NQ = 1280          # fg-query capacity per core (10 tiles of 128)
QT = NQ // 128
NP = NB + NQ       # positions that need feats
PT = NP // 128     # 27
QCH = 256          # attention q-chunk
NCHUNK = NQ // QCH # 5
CCH = 288          # conv position-chunk
NCC = NP // CCH    # 12
CI = 1024          # conv contraction (256 + 768)
C = 256            # channels
KG = 4             # k-tiles per score-psum group ([128, KG*QCH] <= 2 banks,
                   # each matmul inside one bank)

MAX_WAITS = 1

_wsplit_ctr = [0]


def _split_waits(nc, max_waits=MAX_WAITS):
    """This walrus build rejects instructions carrying more than one sync
    wait.  Hoist excess waits onto same-engine NoOps placed just before the
    instruction (same engine stream => identical semantics)."""
    for bbb in nc.bb_map.values():
        bb = bbb.bb
        out = []
        for inst in bb.instructions:
            si = inst.sync_info
            if si is not None and si.on_wait and len(si.on_wait) > max_waits:
                waits = list(si.on_wait)
                k = 0
                while len(waits) - k > max_waits:
                    _wsplit_ctr[0] += 1
                    nop = mybir.InstNoOp(
                        name=f"WSPL-{_wsplit_ctr[0]}", ins=[], outs=[]
                    )
                    nop.engine = inst.engine
                    nop.sync_info = mybir.SyncInfo(
                        on_wait=waits[k : k + max_waits], on_update=[]
                    )
                    out.append(nop)
                    k += max_waits
                inst.sync_info = mybir.SyncInfo(
                    on_wait=waits[k:], on_update=list(si.on_update or [])
                )
            out.append(inst)
        bb.instructions = out


def _bcast_ap(dram_ap, parts=128):
    """DRAM row -> [parts, ...] partition-broadcast source AP."""
    return bass.AP(
        tensor=dram_ap.tensor,
        offset=dram_ap.offset,
        ap=[[0, parts]] + [list(d) for d in dram_ap.ap],
    )


def _build_nc():
    nc = bass.Bass("TRN2", target_bir_lowering=False, debug=False, num_devices=8)

    xcat_d = nc.dram_tensor("xcat", [2, 128, NP], F32R, kind="ExternalInput")
    xdvt_d = nc.dram_tensor("xdvt", [6, 128, NP], BF16, kind="ExternalInput")
    kwT_d = nc.dram_tensor("kwT", [2, 128, C], F32R, kind="ExternalInput")
    kwTb_d = nc.dram_tensor("kwTb", [6, 128, C], BF16, kind="ExternalInput")
    kb_d = nc.dram_tensor("kb", [2, 128], F32, kind="ExternalInput")
    vT_d = nc.dram_tensor("vT", [KT, 128, C], F32R, kind="ExternalInput")
    fwT_d = nc.dram_tensor("fwT", [4, 128, C], F32R, kind="ExternalInput")
    xfg_d = nc.dram_tensor("xfg", [2, 128, NQ], F32R, kind="ExternalInput")
    kqm_d = nc.dram_tensor("kqm", [PT, 128], F32, kind="ExternalInput")
    qvn_d = nc.dram_tensor("qvn", [NQ], F32, kind="ExternalInput")
    padk_d = nc.dram_tensor("padk", [1], F32, kind="ExternalInput")

    fused_d = nc.dram_tensor("fused", [2, 128, NQ], F32, kind="ExternalOutput")
    vis_d = nc.dram_tensor("vis", [128, KT], F32, kind="ExternalOutput")

    # DRAM scratch for partition-broadcast bounces
    rk_d = nc.dram_tensor("rk_scratch", [NP], F32, kind="Internal")
    rq_d = nc.dram_tensor("rq_scratch", [NQ], F32, kind="Internal")
    rden_d = nc.dram_tensor("rden_scratch", [NQ], F32, kind="Internal")
    nrm_d = nc.dram_tensor("nrm_scratch", [NP], F32, kind="Internal")

    with TileContext(nc) as tc:
        with (
            tc.tile_pool(name="res", bufs=1) as res,       # resident tensors
            tc.tile_pool(name="xin", bufs=2) as xin,       # conv input stream
            tc.tile_pool(name="sqp", bufs=2) as sqp,       # squares scratch
            tc.tile_pool(name="expp", bufs=3) as expp,     # exp tiles
            tc.tile_pool(name="attp", bufs=1) as attp,     # bf16 att tiles
            tc.tile_pool(name="smal", bufs=2) as smal,     # small per-chunk tiles
            tc.tile_pool(name="outp", bufs=3) as outp,
            tc.tile_pool(name="catp", bufs=1) as catp,     # output staging
            tc.tile_pool(name="ps", bufs=2, space="PSUM") as psp,
            tc.tile_pool(name="psf", bufs=3, space="PSUM") as psfp,
            tc.tile_pool(name="psd", bufs=1, space="PSUM") as psdp,
        ):
            # ---------- resident loads ----------
            kwT = res.tile([128, 2, C], F32R, tag="kwT")
            nc.scalar.dma_start(out=kwT, in_=kwT_d.ap().rearrange("t p c -> p t c"))
            kwTb = res.tile([128, 6, C], BF16, tag="kwTb")
            nc.scalar.dma_start(out=kwTb, in_=kwTb_d.ap().rearrange("t p c -> p t c"))
            kb = res.tile([128, 2], F32, tag="kb")
            nc.scalar.dma_start(out=kb, in_=kb_d.ap().rearrange("t p -> p t"))
            vT = res.tile([128, KT, C], F32R, tag="vT")
            fwT = res.tile([128, 4, C], F32R, tag="fwT")
            xfg = res.tile([128, 2, NQ], F32R, tag="xfg")
            kqm = res.tile([128, PT], F32, tag="kqm")
            qvn = res.tile([1, NQ], F32, tag="qvn")
            padk = res.tile([1, 1], F32, tag="padk")
            # loaded on the scalar-engine HWDGE queue, traced after conv starts

            ones_f = res.tile([128, 1], F32, tag="ones_f")
            nc.vector.memset(ones_f, 1.0)
            ones_r = res.tile([128, 1], F32R, tag="ones_r")
            nc.vector.tensor_copy(ones_r, ones_f)

            # feats doubles as Kn|Qn after in-place normalization
            feats = res.tile([128, 2, NP], F32R, tag="feats")

            # ---------- phase 1: 1x1 conv  feats = K_w @ xcat + K_b ----------
            xcat_r = xcat_d.ap().rearrange("t p n -> p t n")
            xdvt_r = xdvt_d.ap().rearrange("t p n -> p t n")
            ss128 = res.tile([128, NP], F32R, tag="ss128")
            nrow = res.tile([1, NP], F32, tag="nrow")
            nrm = res.tile([128, PT], F32, tag="nrm")
            r0 = res.tile([128, PT], F32, tag="r0")
            tN = res.tile([128, PT], F32, tag="tN")
            r128 = res.tile([128, PT], F32, tag="r128")
            rall = res.tile([128, NP], F32, tag="rall")
            rqb = rall[:, 0:NQ]
            rkb = rall[:, NQ:NP]

            def emit_norm_tail(t0, t1):
                """recip/Newton/mask + broadcast for position tiles [t0, t1)."""
                p0, p1 = t0 * 128, t1 * 128
                tsl = bass.ds(t0, t1 - t0)
                nc.sync.dma_start(
                    out=nrm_d.ap()[bass.ds(p0, p1 - p0)],
                    in_=nrow[0:1, bass.ds(p0, p1 - p0)],
                )
                nc.sync.dma_start(
                    out=nrm[:, tsl],
                    in_=nrm_d.ap()[bass.ds(p0, p1 - p0)].rearrange(
                        "(t p) -> p t", p=128
                    ),
                )
                nc.vector.tensor_scalar_add(nrm[:, tsl], nrm[:, tsl], 1e-8)
                nc.vector.reciprocal(r0[:, tsl], nrm[:, tsl])
                nc.vector.tensor_mul(tN[:, tsl], nrm[:, tsl], r0[:, tsl])
                nc.vector.tensor_scalar_sub(tN[:, tsl], tN[:, tsl], 2.0)
                nc.vector.tensor_mul(tN[:, tsl], r0[:, tsl], tN[:, tsl])
                nc.vector.tensor_mul(r128[:, tsl], tN[:, tsl], kqm[:, tsl])
                nc.sync.dma_start(
                    out=rk_d.ap()[bass.ds(p0, p1 - p0)].rearrange(
                        "(t p) -> p t", p=128
                    ),
                    in_=r128[:, tsl],
                )
                # broadcast: K-part on sync, Q-part on scalar
                qq0, qq1 = min(p0, NQ), min(p1, NQ)
                if qq1 > qq0:
                    nc.scalar.dma_start(
                        out=rall[:, bass.ds(qq0, qq1 - qq0)],
                        in_=_bcast_ap(rk_d.ap()[bass.ds(qq0, qq1 - qq0)]),
                    )
                kk0, kk1 = max(p0, NQ), max(p1, NQ)
                if kk1 > kk0:
                    nc.sync.dma_start(
                        out=rall[:, bass.ds(kk0, kk1 - kk0)],
                        in_=_bcast_ap(rk_d.ap()[bass.ds(kk0, kk1 - kk0)]),
                    )
                    # normalize the Kn columns this piece covers
                    for ct in range(2):
                        nc.vector.tensor_mul(
                            feats[:, ct, bass.ds(kk0, kk1 - kk0)],
                            feats[:, ct, bass.ds(kk0, kk1 - kk0)].bitcast(F32),
                            rall[:, bass.ds(kk0, kk1 - kk0)],
                        )

            NORM_SPLIT = 13  # tile where the two pipelined norm halves meet
            for cc in range(NCC):
                sl = bass.ts(cc, CCH)
                xt = xin.tile([128, 2, CCH], F32R, tag="xt")
                xtd = xin.tile([128, 6, CCH], BF16, tag="xtd")
                dmae = nc.sync if cc % 2 == 0 else nc.scalar
                dmae2 = nc.scalar if cc % 2 == 0 else nc.sync
                dmae.dma_start(out=xt, in_=xcat_r[:, :, sl])
                dmae2.dma_start(out=xtd, in_=xdvt_r[:, :, sl])
                if cc == 1:
                    # resident loads for later phases; scalar queue, traced here
                    # so they don't delay the first conv chunks
                    nc.scalar.dma_start(out=vT, in_=vT_d.ap().rearrange("t p c -> p t c"))
                    nc.scalar.dma_start(out=fwT, in_=fwT_d.ap().rearrange("t p c -> p t c"))
                    nc.scalar.dma_start(out=xfg, in_=xfg_d.ap().rearrange("t p q -> p t q"))
                    nc.scalar.dma_start(out=kqm, in_=kqm_d.ap().rearrange("t p -> p t"))
                    nc.scalar.dma_start(out=qvn, in_=qvn_d.ap().rearrange("(o q) -> o q", o=1))
                    nc.scalar.dma_start(out=padk, in_=padk_d.ap().rearrange("(o q) -> o q", o=1))
                for co in range(2):
                    ps = psp.tile([128, CCH], F32, tag="ps")
                    for ci in range(8):
                        rhs = xt[:, ci, :] if ci < 2 else xtd[:, ci - 2, :]
                        lhs = (kwT[:, ci, bass.ts(co, 128)] if ci < 2
                               else kwTb[:, ci - 2, bass.ts(co, 128)])
                        nc.tensor.matmul(
                            ps[:],
                            lhs,
                            rhs,
                            start=(ci == 0),
                            stop=(ci == 7),
                        )
                    # psum -> sbuf with per-channel bias (rounds to f32r);
                    # ACT, so the DVE queue stays empty during the conv and the
                    # latency-bound norm tail can't block conv psum drains
                    nc.scalar.activation(
                        out=feats[:, co, sl], in_=ps[:], func=AF.Identity,
                        bias=kb[:, co : co + 1],
                    )
                # per-chunk squares + partition-fold + sqrt so the whole norm
                # reduction overlaps the conv stream
                sq0 = sqp.tile([128, CCH], F32, tag="sq0")
                sq1 = sqp.tile([128, CCH], F32, tag="sq1")
                nc.scalar.activation(out=sq0, in_=feats[:, 0, sl].bitcast(F32), func=AF.Square)
                nc.scalar.activation(out=sq1, in_=feats[:, 1, sl].bitcast(F32), func=AF.Square)
                nc.gpsimd.tensor_add(ss128[:, sl], sq0, sq1)
                psn = psdp.tile([1, CCH], F32, tag="psd")
                nc.tensor.matmul(psn[:], ones_r[:], ss128[:, sl], start=True, stop=True)
                nc.scalar.activation(out=nrow[:, sl], in_=psn[:], func=AF.Sqrt)
                if cc % 4 == 3:
                    tp = (cc + 1) * CCH // 128
                    emit_norm_tail(tp - 9, tp)

            # ---------- phase 2: Kn normalization (emitted inside the conv
            # loop via emit_norm_tail; nothing left to do here) ----------

            # ---------- phase 3: attention, chunked over queries ----------
            vp = res.tile([128, KT, NCHUNK], F32, tag="vp")
            ktg = [list(range(g, min(g + KG, KT))) for g in range(0, KT, KG)]

            def emit_qn(qc):
                qsl = bass.ds(qc * QCH, QCH)
                qslo = bass.ts(qc, QCH)
                for ct in range(2):
                    nc.vector.tensor_mul(
                        feats[:, ct, qsl], feats[:, ct, qsl].bitcast(F32), rqb[:, qslo]
                    )

            def emit_score_exp(qc):
                qsl = bass.ds(qc * QCH, QCH)
                exp_sb = expp.tile([128, KT, QCH], F32R, tag="exp")
                for grp in ktg:
                    ps = psp.tile([128, len(grp) * QCH], F32, tag="ps")
                    for i, kt in enumerate(grp):
                        for ct in range(2):
                            nc.tensor.matmul(
                                ps[:, bass.ts(i, QCH)],
                                feats[:, ct, bass.ds(NQ + kt * 128, 128)],
                                feats[:, ct, qsl],
                                start=(ct == 0),
                                stop=(ct == 1),
                            )
                    nc.scalar.activation(
                        out=exp_sb[:, grp[0] : grp[-1] + 1, :],
                        in_=ps[:],
                        func=AF.Exp,
                        scale=20.0,
                    )
                return exp_sb

            catqs = []
            emit_qn(0)
            for qc in range(NCHUNK):
                qslo = bass.ts(qc, QCH)
                exp_sb = emit_score_exp(qc)
                if qc + 1 < NCHUNK:
                    emit_qn(qc + 1)   # next chunk's Qn ahead of this chunk's DVE tail
                # denom[q] = sum_k exp - n_pad_k
                psd = psdp.tile([1, QCH], F32, tag="psd")
                for kt in range(KT):
                    nc.tensor.matmul(
                        psd[:],
                        ones_r[:],
                        exp_sb[:, kt, :],
                        start=(kt == 0),
                        stop=(kt == KT - 1),
                    )
                den = smal.tile([1, QCH], F32, tag="den")
                nc.vector.tensor_scalar_sub(den, psd[:], padk[0:1, 0:1])
                # rden = qvalid / denom   (Newton-refined; qvn = -1 valid / 0 pad)
                rd0 = smal.tile([1, QCH], F32, tag="rd0")
                nc.vector.reciprocal(rd0, den)
                tD = smal.tile([1, QCH], F32, tag="tD")
                nc.vector.tensor_mul(tD, den, rd0)
                nc.vector.tensor_scalar_sub(tD, tD, 2.0)
                nc.vector.tensor_mul(tD, rd0, tD)      # = -refined recip
                rden = smal.tile([1, QCH], F32, tag="rden")
                nc.vector.tensor_mul(rden, tD, qvn[0:1, qslo])
                # broadcast across partitions via DRAM bounce
                nc.scalar.dma_start(out=rden_d.ap()[qslo], in_=rden)
                rdenb = smal.tile([128, QCH], F32, tag="rdenb")
                nc.scalar.dma_start(out=rdenb, in_=_bcast_ap(rden_d.ap()[qslo]))

                # att (bf16) for visatt: att = exp * rden ; vis += sum_q att
                for half, (k0, k1) in enumerate(((0, 9), (9, KT))):
                    natt = k1 - k0
                    att = attp.tile([128, 9, QCH], BF16, tag="att")
                    rdb3 = bass.AP(
                        tensor=rdenb.tensor,
                        offset=rdenb.offset,
                        ap=[list(rdenb.ap[0]), [0, natt], list(rdenb.ap[1])],
                    )
                    muleng = nc.vector if half == 0 else nc.gpsimd
                    muleng.tensor_mul(
                        att[:, 0:natt, :],
                        exp_sb[:, k0:k1, :].bitcast(F32),
                        rdb3,
                    )
                    nc.vector.reduce_sum(
                        vp[:, k0:k1, qc : qc + 1],
                        att[:, 0:natt, :],
                        axis=mybir.AxisListType.X,
                    )

                # fore = V @ exp, then scale by rden -> catq
                catq = catp.tile([128, 2, QCH], F32R, tag=f"catq{qc}")
                for cm in range(2):
                    psf = psfp.tile([128, QCH], F32, tag="psf")
                    for kt in range(KT):
                        nc.tensor.matmul(
                            psf[:],
                            vT[:, kt, bass.ts(cm, 128)],
                            exp_sb[:, kt, :],
                            start=(kt == 0),
                            stop=(kt == KT - 1),
                        )
                    nc.vector.tensor_mul(catq[:, cm, :], psf[:], rdenb)

                catqs.append(catq)

            # ---------- phase 4: fuse convs (deferred off the chunk chain) ----------
            for qc in range(NCHUNK):
                qslo = bass.ts(qc, QCH)
                catq = catqs[qc]
                for co in range(2):
                    psz = psfp.tile([128, QCH], F32, tag="psf")
                    for ci in range(4):
                        rhs = catq[:, ci, :] if ci < 2 else xfg[:, ci - 2, qslo]
                        nc.tensor.matmul(
                            psz[:],
                            fwT[:, ci, bass.ts(co, 128)],
                            rhs,
                            start=(ci == 0),
                            stop=(ci == 3),
                        )
                    fz = outp.tile([128, QCH], F32, tag="fz")
                    nc.scalar.copy(out=fz, in_=psz[:])
                    nc.scalar.dma_start(out=fused_d.ap()[co, :, qslo], in_=fz)

            # ---------- phase 5: visatt out ----------
            vis = res.tile([128, KT], F32, tag="vis")
            nc.vector.reduce_sum(vis, vp, axis=mybir.AxisListType.X)
            nc.sync.dma_start(out=vis_d.ap(), in_=vis)

    _split_waits(nc)
    return nc


_NC_CACHE = []
_LAST_RESULTS = None


def kernel(x, mask, dvtfeats, K_w, K_b, fuse_w):
    x = np.ascontiguousarray(np.asarray(x, dtype=np.float32))
    mask = np.asarray(mask, dtype=np.float32)
    dvtfeats = np.ascontiguousarray(np.asarray(dvtfeats, dtype=np.float32))
    K_w = np.asarray(K_w, dtype=np.float32)
    K_b = np.asarray(K_b, dtype=np.float32)
    fuse_w = np.asarray(fuse_w, dtype=np.float32)

    b, c, h, w = x.shape
    n = h * w
    assert (b, c, h, w) == (4, 256, 64, 64) and mask.shape == (4, 1, 512, 512)

    # host: maxpool8 + binarize (tiny), fg/bg index lists
    m = (mask.reshape(b, 1, h, 8, w, 8).max(axis=(3, 5)) > 0).astype(np.float32)
    mf = m.reshape(b, n)
    x2 = x.reshape(b, c, n)
    d2 = dvtfeats.reshape(b, dvtfeats.shape[1], n)

    kwT_full = K_w.T.reshape(8, 128, C)
    kwT_h = np.ascontiguousarray(kwT_full[:2])
    kwTb_h = np.ascontiguousarray(kwT_full[2:]).astype(ml_dtypes.bfloat16)
    kb_h = np.ascontiguousarray(K_b.reshape(2, 128))
    fwT_h = np.ascontiguousarray(fuse_w.T.reshape(4, 128, C))

    in_maps = []
    meta = []
    for s in range(b):
        fg = np.nonzero(mf[s] > 0)[0]
        bg = np.nonzero(mf[s] == 0)[0]
        n_fg, n_bg = len(fg), len(bg)
        assert n_bg <= NB, f"n_bg={n_bg} exceeds capacity {NB}"
        xcat_full = np.concatenate([x2[s], d2[s]], axis=0)  # [1024, 4096]

        vT_h = np.zeros((NB, C), np.float32)
        vT_h[:n_bg] = x2[s][:, bg].T
        vT_h = np.ascontiguousarray(vT_h.reshape(KT, 128, C))

        nh = (n_fg + 1) // 2
        for half in range(2):
            fgh = fg[:nh] if half == 0 else fg[nh:]
            nq = len(fgh)
            assert nq <= NQ, f"n_fg_half={nq} exceeds capacity {NQ}"

            xcat_h = np.zeros((CI, NP), np.float32)
            xcat_h[:, :nq] = xcat_full[:, fgh]
            xcat_h[:, NQ : NQ + n_bg] = xcat_full[:, bg]
            xdvt_h = xcat_h[C:].astype(ml_dtypes.bfloat16)

            xfg_h = np.zeros((C, NQ), np.float32)
            xfg_h[:, :nq] = x2[s][:, fgh]

            kqm_h = np.zeros((NP,), np.float32)
            kqm_h[:nq] = -1.0
            kqm_h[NQ : NQ + n_bg] = -1.0

            qvn_h = np.zeros((NQ,), np.float32)
            qvn_h[:nq] = -1.0

            in_maps.append(
                {
                    "xcat": np.ascontiguousarray(xcat_h[:C].reshape(2, 128, NP)),
                    "xdvt": np.ascontiguousarray(xdvt_h.reshape(6, 128, NP)),
                    "kwT": kwT_h,
                    "kwTb": kwTb_h,
                    "kb": kb_h,
                    "vT": vT_h,
                    "fwT": fwT_h,
                    "xfg": np.ascontiguousarray(xfg_h.reshape(2, 128, NQ)),
                    "kqm": np.ascontiguousarray(kqm_h.reshape(PT, 128)),
                    "qvn": qvn_h,
                    "padk": np.array([float(NB - n_bg)], np.float32),
                }
            )
            meta.append((s, fgh, bg, n_bg))

    if not _NC_CACHE:
        _NC_CACHE.append(_build_nc())
    nc = _NC_CACHE[0]
    import os as _os
    _trace = bool(_os.environ.get("KERNEL_TRACE"))
    res = run_bass_kernel_spmd(nc, in_maps, list(range(8)), trace=_trace)
    global _LAST_RESULTS
    _LAST_RESULTS = res

    out = x.reshape(b, c, n).copy()
    visatt = np.zeros((b, n), np.float32)
    for core, (s, fgh, bg, n_bg) in enumerate(meta):
        r = res.results[core]
        fused = r["fused"].reshape(C, NQ)
        if len(fgh):
            out[s][:, fgh] = fused[:, : len(fgh)]
        vis_core = r["vis"].T.reshape(-1)[:n_bg]  # [128, KT] -> pos t*128+p
        visatt[s][bg] += vis_core

    attmask = np.repeat(np.repeat(visatt.reshape(b, 1, h, w), 8, axis=2), 8, axis=3)
    attmask = attmask / attmask.max()
    return out.reshape(b, c, h, w), attmask


# revision 36
# speedup vs baseline: 1.0089x; 1.0089x over previous
"""Trainium2 Bass kernel for the DVT sparse-attention module.

Strategy (8 NeuronCores, data-parallel):
  core = 2*s + h handles sample s (of 4) and half h (of 2) of its
  foreground queries.  The attention is *sparse*: queries are foreground
  positions (maxpool8(mask) > 0), keys are background positions, so each
  per-sample attention is only ~2048 x ~2048 instead of 4096 x 4096.
  The host compacts (gathers) fg/bg columns, the device runs
    feats = K_w @ [x; dvt] + K_b            (1x1 conv, contract 1024)
    Kn,Qn = column-normalized feats          (cosine prep)
    e     = exp(20 * Kn^T Qn)                (scores, [k, q] layout)
    denom = ones^T e ;  att = e / denom
    fore  = V e / denom ;  fused = fuse_w @ [fore; x_fg]
    vis   = sum_q att
  and the host scatters fused back into x (background positions pass
  through x unchanged) and assembles attmask from vis.

Everything runs in float32r on the PE (1 cycle/row for N>=256); exp /
normalization are fp32 with a Newton-refined reciprocal.
"""

import sys

sys.path.insert(0, "/opt/trn_rl_repo")

import numpy as np
import ml_dtypes  # noqa: F401  (bf16 numpy support)

import concourse.bass as bass
import concourse.mybir as mybir
from concourse.tile import TileContext
from concourse.bass_utils import run_bass_kernel_spmd

F32 = mybir.dt.float32
F32R = mybir.dt.float32r
BF16 = mybir.dt.bfloat16
AF = mybir.ActivationFunctionType
ALU = mybir.AluOpType

# capacities (inputs are deterministic; actual counts are ~2061 bg / ~1042 fg-half)
NB = 2176          # background-key capacity   (17 tiles of 128)
KT = NB // 128
NQ = 1280          # fg-query capacity per core (10 tiles of 128)
QT = NQ // 128
NP = NB + NQ       # positions that need feats
PT = NP // 128     # 27
QCH = 256          # attention q-chunk
NCHUNK = NQ // QCH # 5
CCH = 288          # conv position-chunk
NCC = NP // CCH    # 12
CI = 1024          # conv contraction (256 + 768)
C = 256            # channels
KG = 4             # k-tiles per score-psum group ([128, KG*QCH] <= 2 banks,
                   # each matmul inside one bank)

MAX_WAITS = 1

_wsplit_ctr = [0]


def _split_waits(nc, max_waits=MAX_WAITS):
    """This walrus build rejects instructions carrying more than one sync
    wait.  Hoist excess waits onto same-engine NoOps placed just before the
    instruction (same engine stream => identical semantics)."""
    for bbb in nc.bb_map.values():
        bb = bbb.bb
        out = []
        for inst in bb.instructions:
            si = inst.sync_info
            if si is not None and si.on_wait and len(si.on_wait) > max_waits:
                waits = list(si.on_wait)
                k = 0
                while len(waits) - k > max_waits:
                    _wsplit_ctr[0] += 1
                    nop = mybir.InstNoOp(
                        name=f"WSPL-{_wsplit_ctr[0]}", ins=[], outs=[]
                    )
                    nop.engine = inst.engine
                    nop.sync_info = mybir.SyncInfo(
                        on_wait=waits[k : k + max_waits], on_update=[]
                    )
                    out.append(nop)
                    k += max_waits
                inst.sync_info = mybir.SyncInfo(
                    on_wait=waits[k:], on_update=list(si.on_update or [])
                )
            out.append(inst)
        bb.instructions = out


def _bcast_ap(dram_ap, parts=128):
    """DRAM row -> [parts, ...] partition-broadcast source AP."""
    return bass.AP(
        tensor=dram_ap.tensor,
        offset=dram_ap.offset,
        ap=[[0, parts]] + [list(d) for d in dram_ap.ap],
    )


def _build_nc():
    nc = bass.Bass("TRN2", target_bir_lowering=False, debug=False, num_devices=8)

    xcat_d = nc.dram_tensor("xcat", [2, 128, NP], F32R, kind="ExternalInput")
    xdvt_d = nc.dram_tensor("xdvt", [6, 128, NP], BF16, kind="ExternalInput")
    kwT_d = nc.dram_tensor("kwT", [2, 128, C], F32R, kind="ExternalInput")
    kwTb_d = nc.dram_tensor("kwTb", [6, 128, C], BF16, kind="ExternalInput")
    kb_d = nc.dram_tensor("kb", [2, 128], F32, kind="ExternalInput")
    vT_d = nc.dram_tensor("vT", [KT, 128, C], F32R, kind="ExternalInput")
    fwT_d = nc.dram_tensor("fwT", [4, 128, C], F32R, kind="ExternalInput")
    xfg_d = nc.dram_tensor("xfg", [2, 128, NQ], F32R, kind="ExternalInput")
    kqm_d = nc.dram_tensor("kqm", [PT, 128], F32, kind="ExternalInput")
    qvn_d = nc.dram_tensor("qvn", [NQ], F32, kind="ExternalInput")
    padk_d = nc.dram_tensor("padk", [1], F32, kind="ExternalInput")

    fused_d = nc.dram_tensor("fused", [2, 128, NQ], F32, kind="ExternalOutput")
    vis_d = nc.dram_tensor("vis", [128, KT], F32, kind="ExternalOutput")

    # DRAM scratch for partition-broadcast bounces
    rk_d = nc.dram_tensor("rk_scratch", [NP], F32, kind="Internal")
    rq_d = nc.dram_tensor("rq_scratch", [NQ], F32, kind="Internal")
    rden_d = nc.dram_tensor("rden_scratch", [NQ], F32, kind="Internal")
    nrm_d = nc.dram_tensor("nrm_scratch", [NP], F32, kind="Internal")

    with TileContext(nc) as tc:
        with (
            tc.tile_pool(name="res", bufs=1) as res,       # resident tensors
            tc.tile_pool(name="xin", bufs=2) as xin,       # conv input stream
            tc.tile_pool(name="sqp", bufs=2) as sqp,       # squares scratch
            tc.tile_pool(name="expp", bufs=3) as expp,     # exp tiles
            tc.tile_pool(name="attp", bufs=1) as attp,     # bf16 att tiles
            tc.tile_pool(name="smal", bufs=2) as smal,     # small per-chunk tiles
            tc.tile_pool(name="outp", bufs=3) as outp,
            tc.tile_pool(name="catp", bufs=1) as catp,     # output staging
            tc.tile_pool(name="ps", bufs=2, space="PSUM") as psp,
            tc.tile_pool(name="psf", bufs=2, space="PSUM") as psfp,
            tc.tile_pool(name="psd", bufs=1, space="PSUM") as psdp,
        ):
            # ---------- resident loads ----------
            kwT = res.tile([128, 2, C], F32R, tag="kwT")
            nc.scalar.dma_start(out=kwT, in_=kwT_d.ap().rearrange("t p c -> p t c"))
            kwTb = res.tile([128, 6, C], BF16, tag="kwTb")
            nc.scalar.dma_start(out=kwTb, in_=kwTb_d.ap().rearrange("t p c -> p t c"))
            kb = res.tile([128, 2], F32, tag="kb")
            nc.scalar.dma_start(out=kb, in_=kb_d.ap().rearrange("t p -> p t"))
            vT = res.tile([128, KT, C], F32R, tag="vT")
            fwT = res.tile([128, 4, C], F32R, tag="fwT")
            xfg = res.tile([128, 2, NQ], F32R, tag="xfg")
            kqm = res.tile([128, PT], F32, tag="kqm")
            qvn = res.tile([1, NQ], F32, tag="qvn")
            padk = res.tile([1, 1], F32, tag="padk")
            # loaded on the scalar-engine HWDGE queue, traced after conv starts

            ones_f = res.tile([128, 1], F32, tag="ones_f")
            nc.vector.memset(ones_f, 1.0)
            ones_r = res.tile([128, 1], F32R, tag="ones_r")
            nc.vector.tensor_copy(ones_r, ones_f)
            ones_rowf = res.tile([1, 128], F32, tag="ones_rowf")
            nc.vector.memset(ones_rowf, 1.0)
            ones_row = res.tile([1, 128], F32R, tag="ones_row")
            nc.vector.tensor_copy(ones_row, ones_rowf)

            # feats doubles as Kn|Qn after in-place normalization
            feats = res.tile([128, 2, NP], F32R, tag="feats")

            # ---------- phase 1: 1x1 conv  feats = K_w @ xcat + K_b ----------
            xcat_r = xcat_d.ap().rearrange("t p n -> p t n")
            xdvt_r = xdvt_d.ap().rearrange("t p n -> p t n")
            ss128 = res.tile([128, NP], F32R, tag="ss128")
            nrow = res.tile([1, NP], F32, tag="nrow")
            nrm = res.tile([128, PT], F32, tag="nrm")
            r0 = res.tile([128, PT], F32, tag="r0")
            tN = res.tile([128, PT], F32, tag="tN")
            r128 = res.tile([128, PT], F32, tag="r128")
            rall = res.tile([128, NP], F32, tag="rall")
            rqb = rall[:, 0:NQ]
            rkb = rall[:, NQ:NP]

            def emit_norm_tail(t0, t1):
                """recip/Newton/mask + broadcast for position tiles [t0, t1)."""
                p0, p1 = t0 * 128, t1 * 128
                tsl = bass.ds(t0, t1 - t0)
                nc.sync.dma_start(
                    out=nrm_d.ap()[bass.ds(p0, p1 - p0)],
                    in_=nrow[0:1, bass.ds(p0, p1 - p0)],
                )
                nc.sync.dma_start(
                    out=nrm[:, tsl],
                    in_=nrm_d.ap()[bass.ds(p0, p1 - p0)].rearrange(
                        "(t p) -> p t", p=128
                    ),
                )
                nc.vector.tensor_scalar_add(nrm[:, tsl], nrm[:, tsl], 1e-8)
                nc.vector.reciprocal(r0[:, tsl], nrm[:, tsl])
                nc.vector.tensor_mul(tN[:, tsl], nrm[:, tsl], r0[:, tsl])
                nc.vector.tensor_scalar_sub(tN[:, tsl], tN[:, tsl], 2.0)
                nc.vector.tensor_mul(tN[:, tsl], r0[:, tsl], tN[:, tsl])
                nc.vector.tensor_mul(r128[:, tsl], tN[:, tsl], kqm[:, tsl])
                nc.sync.dma_start(
                    out=rk_d.ap()[bass.ds(p0, p1 - p0)].rearrange(
                        "(t p) -> p t", p=128
                    ),
                    in_=r128[:, tsl],
                )
                # broadcast: K-part on sync, Q-part on scalar
                qq0, qq1 = min(p0, NQ), min(p1, NQ)
                if qq1 > qq0:
                    nc.scalar.dma_start(
                        out=rall[:, bass.ds(qq0, qq1 - qq0)],
                        in_=_bcast_ap(rk_d.ap()[bass.ds(qq0, qq1 - qq0)]),
                    )
                kk0, kk1 = max(p0, NQ), max(p1, NQ)
                if kk1 > kk0:
                    nc.sync.dma_start(
                        out=rall[:, bass.ds(kk0, kk1 - kk0)],
                        in_=_bcast_ap(rk_d.ap()[bass.ds(kk0, kk1 - kk0)]),
                    )
                    # normalize the Kn columns this piece covers
                    for ct in range(2):
                        nc.vector.tensor_mul(
                            feats[:, ct, bass.ds(kk0, kk1 - kk0)],
                            feats[:, ct, bass.ds(kk0, kk1 - kk0)].bitcast(F32),
                            rall[:, bass.ds(kk0, kk1 - kk0)],
                        )

            NORM_SPLIT = 13  # tile where the two pipelined norm halves meet
            for cc in range(NCC):
                sl = bass.ts(cc, CCH)
                xt = xin.tile([128, 2, CCH], F32R, tag="xt")
                xtd = xin.tile([128, 6, CCH], BF16, tag="xtd")
                dmae = nc.sync if cc % 2 == 0 else nc.scalar
                dmae2 = nc.scalar if cc % 2 == 0 else nc.sync
                dmae.dma_start(out=xt, in_=xcat_r[:, :, sl])
                dmae2.dma_start(out=xtd, in_=xdvt_r[:, :, sl])
                if cc == 1:
                    # resident loads for later phases; scalar queue, traced here
                    # so they don't delay the first conv chunks
                    nc.scalar.dma_start(out=vT, in_=vT_d.ap().rearrange("t p c -> p t c"))
                    nc.scalar.dma_start(out=fwT, in_=fwT_d.ap().rearrange("t p c -> p t c"))
                    nc.scalar.dma_start(out=xfg, in_=xfg_d.ap().rearrange("t p q -> p t q"))
                    nc.scalar.dma_start(out=kqm, in_=kqm_d.ap().rearrange("t p -> p t"))
                    nc.scalar.dma_start(out=qvn, in_=qvn_d.ap().rearrange("(o q) -> o q", o=1))
                    nc.scalar.dma_start(out=padk, in_=padk_d.ap().rearrange("(o q) -> o q", o=1))
                for co in range(2):
                    ps = psp.tile([128, CCH], F32, tag="ps")
                    for ci in range(8):
                        rhs = xt[:, ci, :] if ci < 2 else xtd[:, ci - 2, :]
                        lhs = (kwT[:, ci, bass.ts(co, 128)] if ci < 2
                               else kwTb[:, ci - 2, bass.ts(co, 128)])
                        nc.tensor.matmul(
                            ps[:],
                            lhs,
                            rhs,
                            start=(ci == 0),
                            stop=(ci == 7),
                        )
                    # psum -> sbuf with per-channel bias (rounds to f32r);
                    # ACT, so the DVE queue stays empty during the conv and the
                    # latency-bound norm tail can't block conv psum drains
                    nc.scalar.activation(
                        out=feats[:, co, sl], in_=ps[:], func=AF.Identity,
                        bias=kb[:, co : co + 1],
                    )
                # per-chunk squares + partition-fold + sqrt so the whole norm
                # reduction overlaps the conv stream
                sq0 = sqp.tile([128, CCH], F32, tag="sq0")
                sq1 = sqp.tile([128, CCH], F32, tag="sq1")
                nc.scalar.activation(out=sq0, in_=feats[:, 0, sl].bitcast(F32), func=AF.Square)
                nc.scalar.activation(out=sq1, in_=feats[:, 1, sl].bitcast(F32), func=AF.Square)
                nc.gpsimd.tensor_add(ss128[:, sl], sq0, sq1)
                psn = psdp.tile([1, CCH], F32, tag="psd")
                nc.tensor.matmul(psn[:], ones_r[:], ss128[:, sl], start=True, stop=True)
                nc.scalar.activation(out=nrow[:, sl], in_=psn[:], func=AF.Sqrt)
                if cc % 4 == 3:
                    tp = (cc + 1) * CCH // 128
                    emit_norm_tail(tp - 9, tp)

            # ---------- phase 2: Kn normalization (emitted inside the conv
            # loop via emit_norm_tail; nothing left to do here) ----------

            # ---------- phase 3: attention, chunked over queries ----------
            vp = res.tile([128, KT, NCHUNK], F32, tag="vp")
            ktg = [list(range(g, min(g + KG, KT))) for g in range(0, KT, KG)]

            def emit_qn(qc):
                qsl = bass.ds(qc * QCH, QCH)
                qslo = bass.ts(qc, QCH)
                for ct in range(2):
                    nc.vector.tensor_mul(
                        feats[:, ct, qsl], feats[:, ct, qsl].bitcast(F32), rqb[:, qslo]
                    )

            def emit_score_exp(qc):
                qsl = bass.ds(qc * QCH, QCH)
                exp_sb = expp.tile([128, KT, QCH], F32R, tag="exp")
                for grp in ktg:
                    ps = psp.tile([128, len(grp) * QCH], F32, tag="ps")
                    for i, kt in enumerate(grp):
                        for ct in range(2):
                            nc.tensor.matmul(
                                ps[:, bass.ts(i, QCH)],
                                feats[:, ct, bass.ds(NQ + kt * 128, 128)],
                                feats[:, ct, qsl],
                                start=(ct == 0),
                                stop=(ct == 1),
                            )
                    nc.scalar.activation(
                        out=exp_sb[:, grp[0] : grp[-1] + 1, :],
                        in_=ps[:],
                        func=AF.Exp,
                        scale=20.0,
                    )
                return exp_sb

            catqs = []
            emit_qn(0)
            for qc in range(NCHUNK):
                qslo = bass.ts(qc, QCH)
                exp_sb = emit_score_exp(qc)
                if qc + 1 < NCHUNK:
                    emit_qn(qc + 1)   # next chunk's Qn ahead of this chunk's DVE tail
                # denom[q] = sum_k exp - n_pad_k
                psd = psdp.tile([1, QCH], F32, tag="psd")
                for kt in range(KT):
                    nc.tensor.matmul(
                        psd[:],
                        ones_r[:],
                        exp_sb[:, kt, :],
                        start=(kt == 0),
                        stop=(kt == KT - 1),
                    )
                den = smal.tile([1, QCH], F32, tag="den")
                nc.vector.tensor_scalar_sub(den, psd[:], padk[0:1, 0:1])
                # rden = qvalid / denom   (Newton-refined; qvn = -1 valid / 0 pad)
                rd0 = smal.tile([1, QCH], F32, tag="rd0")
                nc.vector.reciprocal(rd0, den)
                tD = smal.tile([1, QCH], F32, tag="tD")
                nc.vector.tensor_mul(tD, den, rd0)
                nc.vector.tensor_scalar_sub(tD, tD, 2.0)
                nc.vector.tensor_mul(tD, rd0, tD)      # = -refined recip
                rden = smal.tile([1, QCH], F32R, tag="rden")
                nc.vector.tensor_mul(rden, tD, qvn[0:1, qslo])
                # broadcast across partitions on the PE: ones[1,128].T @ rden
                rdps = psdp.tile([128, QCH], F32, tag="psdb")
                nc.tensor.matmul(rdps[:], ones_row[:], rden[:], start=True, stop=True)
                rdenb = smal.tile([128, QCH], F32, tag="rdenb")
                nc.vector.tensor_copy(rdenb, rdps[:])

                # att (bf16) for visatt: att = exp * rden ; vis += sum_q att
                for half, (k0, k1) in enumerate(((0, 9), (9, KT))):
                    natt = k1 - k0
                    att = attp.tile([128, 9, QCH], BF16, tag="att")
                    rdb3 = bass.AP(
                        tensor=rdenb.tensor,
                        offset=rdenb.offset,
                        ap=[list(rdenb.ap[0]), [0, natt], list(rdenb.ap[1])],
                    )
                    muleng = nc.vector if half == 0 else nc.gpsimd
                    muleng.tensor_mul(
                        att[:, 0:natt, :],
                        exp_sb[:, k0:k1, :].bitcast(F32),
                        rdb3,
                    )
                    nc.vector.reduce_sum(
                        vp[:, k0:k1, qc : qc + 1],
                        att[:, 0:natt, :],
                        axis=mybir.AxisListType.X,
                    )

                # fore = V @ exp, then scale by rden -> catq
                catq = catp.tile([128, 2, QCH], F32R, tag=f"catq{qc}")
                for cm in range(2):
                    psf = psfp.tile([128, QCH], F32, tag="psf")
                    for kt in range(KT):
                        nc.tensor.matmul(
                            psf[:],
                            vT[:, kt, bass.ts(cm, 128)],
                            exp_sb[:, kt, :],
                            start=(kt == 0),
                            stop=(kt == KT - 1),
                        )
                    nc.vector.tensor_mul(catq[:, cm, :], psf[:], rdenb)

                catqs.append(catq)

            # ---------- phase 4: fuse convs (deferred off the chunk chain) ----------
            for qc in range(NCHUNK):
                qslo = bass.ts(qc, QCH)
                catq = catqs[qc]
                for co in range(2):
                    psz = psfp.tile([128, QCH], F32, tag="psf")
                    for ci in range(4):
                        rhs = catq[:, ci, :] if ci < 2 else xfg[:, ci - 2, qslo]
                        nc.tensor.matmul(
                            psz[:],
                            fwT[:, ci, bass.ts(co, 128)],
                            rhs,
                            start=(ci == 0),
                            stop=(ci == 3),
                        )
                    fz = outp.tile([128, QCH], F32, tag="fz")
                    nc.scalar.copy(out=fz, in_=psz[:])
                    nc.scalar.dma_start(out=fused_d.ap()[co, :, qslo], in_=fz)

            # ---------- phase 5: visatt out ----------
            vis = res.tile([128, KT], F32, tag="vis")
            nc.vector.reduce_sum(vis, vp, axis=mybir.AxisListType.X)
            nc.sync.dma_start(out=vis_d.ap(), in_=vis)

    _split_waits(nc)
    return nc


_NC_CACHE = []
_LAST_RESULTS = None


def kernel(x, mask, dvtfeats, K_w, K_b, fuse_w):
    x = np.ascontiguousarray(np.asarray(x, dtype=np.float32))
    mask = np.asarray(mask, dtype=np.float32)
    dvtfeats = np.ascontiguousarray(np.asarray(dvtfeats, dtype=np.float32))
    K_w = np.asarray(K_w, dtype=np.float32)
    K_b = np.asarray(K_b, dtype=np.float32)
    fuse_w = np.asarray(fuse_w, dtype=np.float32)

    b, c, h, w = x.shape
    n = h * w
    assert (b, c, h, w) == (4, 256, 64, 64) and mask.shape == (4, 1, 512, 512)

    # host: maxpool8 + binarize (tiny), fg/bg index lists
    m = (mask.reshape(b, 1, h, 8, w, 8).max(axis=(3, 5)) > 0).astype(np.float32)
    mf = m.reshape(b, n)
    x2 = x.reshape(b, c, n)
    d2 = dvtfeats.reshape(b, dvtfeats.shape[1], n)

    kwT_full = K_w.T.reshape(8, 128, C)
    kwT_h = np.ascontiguousarray(kwT_full[:2])
    kwTb_h = np.ascontiguousarray(kwT_full[2:]).astype(ml_dtypes.bfloat16)
    kb_h = np.ascontiguousarray(K_b.reshape(2, 128))
    fwT_h = np.ascontiguousarray(fuse_w.T.reshape(4, 128, C))

    in_maps = []
    meta = []
    for s in range(b):
        fg = np.nonzero(mf[s] > 0)[0]
        bg = np.nonzero(mf[s] == 0)[0]
        n_fg, n_bg = len(fg), len(bg)
        assert n_bg <= NB, f"n_bg={n_bg} exceeds capacity {NB}"
        xcat_full = np.concatenate([x2[s], d2[s]], axis=0)  # [1024, 4096]

        vT_h = np.zeros((NB, C), np.float32)
        vT_h[:n_bg] = x2[s][:, bg].T
        vT_h = np.ascontiguousarray(vT_h.reshape(KT, 128, C))

        nh = (n_fg + 1) // 2
        for half in range(2):
            fgh = fg[:nh] if half == 0 else fg[nh:]
            nq = len(fgh)
            assert nq <= NQ, f"n_fg_half={nq} exceeds capacity {NQ}"

            xcat_h = np.zeros((CI, NP), np.float32)
            xcat_h[:, :nq] = xcat_full[:, fgh]
            xcat_h[:, NQ : NQ + n_bg] = xcat_full[:, bg]
            xdvt_h = xcat_h[C:].astype(ml_dtypes.bfloat16)

            xfg_h = np.zeros((C, NQ), np.float32)
            xfg_h[:, :nq] = x2[s][:, fgh]

            kqm_h = np.zeros((NP,), np.float32)
            kqm_h[:nq] = -1.0
            kqm_h[NQ : NQ + n_bg] = -1.0

            qvn_h = np.zeros((NQ,), np.float32)
            qvn_h[:nq] = -1.0

            in_maps.append(
                {
                    "xcat": np.ascontiguousarray(xcat_h[:C].reshape(2, 128, NP)),
                    "xdvt": np.ascontiguousarray(xdvt_h.reshape(6, 128, NP)),
                    "kwT": kwT_h,
                    "kwTb": kwTb_h,
                    "kb": kb_h,
                    "vT": vT_h,
                    "fwT": fwT_h,
                    "xfg": np.ascontiguousarray(xfg_h.reshape(2, 128, NQ)),
                    "kqm": np.ascontiguousarray(kqm_h.reshape(PT, 128)),
                    "qvn": qvn_h,
                    "padk": np.array([float(NB - n_bg)], np.float32),
                }
            )
            meta.append((s, fgh, bg, n_bg))

    if not _NC_CACHE:
        _NC_CACHE.append(_build_nc())
    nc = _NC_CACHE[0]
    import os as _os
    _trace = bool(_os.environ.get("KERNEL_TRACE"))
    res = run_bass_kernel_spmd(nc, in_maps, list(range(8)), trace=_trace)
    global _LAST_RESULTS
    _LAST_RESULTS = res

    out = x.reshape(b, c, n).copy()
    visatt = np.zeros((b, n), np.float32)
    for core, (s, fgh, bg, n_bg) in enumerate(meta):
        r = res.results[core]
        fused = r["fused"].reshape(C, NQ)
        if len(fgh):
            out[s][:, fgh] = fused[:, : len(fgh)]
        vis_core = r["vis"].T.reshape(-1)[:n_bg]  # [128, KT] -> pos t*128+p
        visatt[s][bg] += vis_core

    attmask = np.repeat(np.repeat(visatt.reshape(b, 1, h, w), 8, axis=2), 8, axis=3)
    attmask = attmask / attmask.max()
    return out.reshape(b, c, h, w), attmask


# revision 42
# speedup vs baseline: 1.0207x; 1.0117x over previous
"""Trainium2 Bass kernel for the DVT sparse-attention module.

Strategy (8 NeuronCores, data-parallel):
  core = 2*s + h handles sample s (of 4) and half h (of 2) of its
  foreground queries.  The attention is *sparse*: queries are foreground
  positions (maxpool8(mask) > 0), keys are background positions, so each
  per-sample attention is only ~2048 x ~2048 instead of 4096 x 4096.
  The host compacts (gathers) fg/bg columns, the device runs
    feats = K_w @ [x; dvt] + K_b            (1x1 conv, contract 1024)
    Kn,Qn = column-normalized feats          (cosine prep)
    e     = exp(20 * Kn^T Qn)                (scores, [k, q] layout)
    denom = ones^T e ;  att = e / denom
    fore  = V e / denom ;  fused = fuse_w @ [fore; x_fg]
    vis   = sum_q att
  and the host scatters fused back into x (background positions pass
  through x unchanged) and assembles attmask from vis.

Everything runs in float32r on the PE (1 cycle/row for N>=256); exp /
normalization are fp32 with a Newton-refined reciprocal.
"""

import sys

sys.path.insert(0, "/opt/trn_rl_repo")

import numpy as np
import ml_dtypes  # noqa: F401  (bf16 numpy support)

import concourse.bass as bass
import concourse.mybir as mybir
from concourse.tile import TileContext
from concourse.bass_utils import run_bass_kernel_spmd

F32 = mybir.dt.float32
F32R = mybir.dt.float32r
BF16 = mybir.dt.bfloat16
AF = mybir.ActivationFunctionType
ALU = mybir.AluOpType

# capacities (inputs are deterministic; actual counts are ~2061 bg / ~1042 fg-half)
NB = 2176          # background-key capacity   (17 tiles of 128)
KT = NB // 128
NQ = 1280          # fg-query capacity per core (10 tiles of 128)
QT = NQ // 128
NP = NB + NQ       # positions that need feats
PT = NP // 128     # 27
QCH = 256          # attention q-chunk
NCHUNK = NQ // QCH # 5
CCH = 288          # conv position-chunk
NCC = NP // CCH    # 12
CI = 1024          # conv contraction (256 + 768)
C = 256            # channels
KG = 4             # k-tiles per score-psum group ([128, KG*QCH] <= 2 banks,
                   # each matmul inside one bank)

MAX_WAITS = 1

_wsplit_ctr = [0]


def _split_waits(nc, max_waits=MAX_WAITS):
    """This walrus build rejects instructions carrying more than one sync
    wait.  Hoist excess waits onto same-engine NoOps placed just before the
    instruction (same engine stream => identical semantics)."""
    for bbb in nc.bb_map.values():
        bb = bbb.bb
        out = []
        for inst in bb.instructions:
            si = inst.sync_info
            if si is not None and si.on_wait and len(si.on_wait) > max_waits:
                waits = list(si.on_wait)
                k = 0
                while len(waits) - k > max_waits:
                    _wsplit_ctr[0] += 1
                    nop = mybir.InstNoOp(
                        name=f"WSPL-{_wsplit_ctr[0]}", ins=[], outs=[]
                    )
                    nop.engine = inst.engine
                    nop.sync_info = mybir.SyncInfo(
                        on_wait=waits[k : k + max_waits], on_update=[]
                    )
                    out.append(nop)
                    k += max_waits
                inst.sync_info = mybir.SyncInfo(
                    on_wait=waits[k:], on_update=list(si.on_update or [])
                )
            out.append(inst)
        bb.instructions = out


def _bcast_ap(dram_ap, parts=128):
    """DRAM row -> [parts, ...] partition-broadcast source AP."""
    return bass.AP(
        tensor=dram_ap.tensor,
        offset=dram_ap.offset,
        ap=[[0, parts]] + [list(d) for d in dram_ap.ap],
    )


def _build_nc():
    nc = bass.Bass("TRN2", target_bir_lowering=False, debug=False, num_devices=8)

    xcat_d = nc.dram_tensor("xcat", [2, 128, NP], F32R, kind="ExternalInput")
    xdvt_d = nc.dram_tensor("xdvt", [6, 128, NP], BF16, kind="ExternalInput")
    kwT_d = nc.dram_tensor("kwT", [2, 128, C], F32R, kind="ExternalInput")
    kwTb_d = nc.dram_tensor("kwTb", [6, 128, C], BF16, kind="ExternalInput")
    kb_d = nc.dram_tensor("kb", [2, 128], F32, kind="ExternalInput")
    vT_d = nc.dram_tensor("vT", [KT, 128, C], BF16, kind="ExternalInput")
    fwT_d = nc.dram_tensor("fwT", [4, 128, C], F32R, kind="ExternalInput")
    xfg_d = nc.dram_tensor("xfg", [2, 128, NQ], F32R, kind="ExternalInput")
    kqm_d = nc.dram_tensor("kqm", [PT, 128], F32, kind="ExternalInput")
    qvn_d = nc.dram_tensor("qvn", [NQ], F32, kind="ExternalInput")
    padk_d = nc.dram_tensor("padk", [1], F32, kind="ExternalInput")

    fused_d = nc.dram_tensor("fused", [2, 128, NQ], F32, kind="ExternalOutput")
    vis_d = nc.dram_tensor("vis", [128, KT], F32, kind="ExternalOutput")

    # DRAM scratch for partition-broadcast bounces
    rk_d = nc.dram_tensor("rk_scratch", [NP], F32, kind="Internal")
    rq_d = nc.dram_tensor("rq_scratch", [NQ], F32, kind="Internal")
    rden_d = nc.dram_tensor("rden_scratch", [NQ], F32, kind="Internal")
    nrm_d = nc.dram_tensor("nrm_scratch", [NP], F32, kind="Internal")

    with TileContext(nc) as tc:
        with (
            tc.tile_pool(name="res", bufs=1) as res,       # resident tensors
            tc.tile_pool(name="xin", bufs=2) as xin,       # conv input stream
            tc.tile_pool(name="sqp", bufs=2) as sqp,       # squares scratch
            tc.tile_pool(name="expp", bufs=3) as expp,     # exp tiles
            tc.tile_pool(name="attp", bufs=1) as attp,     # bf16 att tiles
            tc.tile_pool(name="smal", bufs=2) as smal,     # small per-chunk tiles
            tc.tile_pool(name="outp", bufs=3) as outp,
            tc.tile_pool(name="catp", bufs=1) as catp,     # output staging
            tc.tile_pool(name="ps", bufs=2, space="PSUM") as psp,
            tc.tile_pool(name="psf", bufs=2, space="PSUM") as psfp,
            tc.tile_pool(name="psd", bufs=1, space="PSUM") as psdp,
        ):
            # ---------- resident loads ----------
            kwT = res.tile([128, 2, C], F32R, tag="kwT")
            nc.scalar.dma_start(out=kwT, in_=kwT_d.ap().rearrange("t p c -> p t c"))
            kwTb = res.tile([128, 6, C], BF16, tag="kwTb")
            nc.scalar.dma_start(out=kwTb, in_=kwTb_d.ap().rearrange("t p c -> p t c"))
            kb = res.tile([128, 2], F32, tag="kb")
            nc.scalar.dma_start(out=kb, in_=kb_d.ap().rearrange("t p -> p t"))
            vT = res.tile([128, KT, C], BF16, tag="vT")
            fwT = res.tile([128, 4, C], F32R, tag="fwT")
            xfg = res.tile([128, 2, NQ], F32R, tag="xfg")
            kqm = res.tile([128, PT], F32, tag="kqm")
            qvn = res.tile([1, NQ], F32, tag="qvn")
            padk = res.tile([1, 1], F32, tag="padk")
            # loaded on the scalar-engine HWDGE queue, traced after conv starts

            ones_f = res.tile([128, 1], F32, tag="ones_f")
            nc.vector.memset(ones_f, 1.0)
            ones_r = res.tile([128, 1], F32R, tag="ones_r")
            nc.vector.tensor_copy(ones_r, ones_f)
            ones_rb = res.tile([128, 1], BF16, tag="ones_rb")
            nc.vector.tensor_copy(ones_rb, ones_f)
            ones_rowf = res.tile([1, 128], F32, tag="ones_rowf")
            nc.vector.memset(ones_rowf, 1.0)
            ones_row = res.tile([1, 128], F32R, tag="ones_row")
            nc.vector.tensor_copy(ones_row, ones_rowf)

            # feats doubles as Kn|Qn after in-place normalization
            feats = res.tile([128, 2, NP], F32R, tag="feats")

            # ---------- phase 1: 1x1 conv  feats = K_w @ xcat + K_b ----------
            xcat_r = xcat_d.ap().rearrange("t p n -> p t n")
            xdvt_r = xdvt_d.ap().rearrange("t p n -> p t n")
            ss128 = res.tile([128, NP], F32R, tag="ss128")
            nrow = res.tile([1, NP], F32, tag="nrow")
            nrm = res.tile([128, PT], F32, tag="nrm")
            r0 = res.tile([128, PT], F32, tag="r0")
            tN = res.tile([128, PT], F32, tag="tN")
            r128 = res.tile([128, PT], F32, tag="r128")
            rall = res.tile([128, NP], F32, tag="rall")
            rqb = rall[:, 0:NQ]
            rkb = rall[:, NQ:NP]

            def emit_norm_tail(t0, t1):
                """recip/Newton/mask + broadcast for position tiles [t0, t1)."""
                p0, p1 = t0 * 128, t1 * 128
                tsl = bass.ds(t0, t1 - t0)
                nc.sync.dma_start(
                    out=nrm_d.ap()[bass.ds(p0, p1 - p0)],
                    in_=nrow[0:1, bass.ds(p0, p1 - p0)],
                )
                nc.sync.dma_start(
                    out=nrm[:, tsl],
                    in_=nrm_d.ap()[bass.ds(p0, p1 - p0)].rearrange(
                        "(t p) -> p t", p=128
                    ),
                )
                nc.vector.tensor_scalar_add(nrm[:, tsl], nrm[:, tsl], 1e-8)
                nc.vector.reciprocal(r0[:, tsl], nrm[:, tsl])
                nc.vector.tensor_mul(tN[:, tsl], nrm[:, tsl], r0[:, tsl])
                nc.vector.tensor_scalar_sub(tN[:, tsl], tN[:, tsl], 2.0)
                nc.vector.tensor_mul(tN[:, tsl], r0[:, tsl], tN[:, tsl])
                nc.vector.tensor_mul(r128[:, tsl], tN[:, tsl], kqm[:, tsl])
                nc.sync.dma_start(
                    out=rk_d.ap()[bass.ds(p0, p1 - p0)].rearrange(
                        "(t p) -> p t", p=128
                    ),
                    in_=r128[:, tsl],
                )
                # broadcast: K-part on sync, Q-part on scalar
                qq0, qq1 = min(p0, NQ), min(p1, NQ)
                if qq1 > qq0:
                    nc.scalar.dma_start(
                        out=rall[:, bass.ds(qq0, qq1 - qq0)],
                        in_=_bcast_ap(rk_d.ap()[bass.ds(qq0, qq1 - qq0)]),
                    )
                kk0, kk1 = max(p0, NQ), max(p1, NQ)
                if kk1 > kk0:
                    nc.sync.dma_start(
                        out=rall[:, bass.ds(kk0, kk1 - kk0)],
                        in_=_bcast_ap(rk_d.ap()[bass.ds(kk0, kk1 - kk0)]),
                    )
                    # normalize the Kn columns this piece covers
                    for ct in range(2):
                        nc.vector.tensor_mul(
                            feats[:, ct, bass.ds(kk0, kk1 - kk0)],
                            feats[:, ct, bass.ds(kk0, kk1 - kk0)].bitcast(F32),
                            rall[:, bass.ds(kk0, kk1 - kk0)],
                        )

            NORM_SPLIT = 13  # tile where the two pipelined norm halves meet
            for cc in range(NCC):
                sl = bass.ts(cc, CCH)
                xt = xin.tile([128, 2, CCH], F32R, tag="xt")
                xtd = xin.tile([128, 6, CCH], BF16, tag="xtd")
                dmae = nc.sync if cc % 2 == 0 else nc.scalar
                dmae2 = nc.scalar if cc % 2 == 0 else nc.sync
                dmae.dma_start(out=xt, in_=xcat_r[:, :, sl])
                dmae2.dma_start(out=xtd, in_=xdvt_r[:, :, sl])
                if cc == 1:
                    # resident loads for later phases; scalar queue, traced here
                    # so they don't delay the first conv chunks
                    nc.scalar.dma_start(out=vT, in_=vT_d.ap().rearrange("t p c -> p t c"))
                    nc.scalar.dma_start(out=fwT, in_=fwT_d.ap().rearrange("t p c -> p t c"))
                    nc.scalar.dma_start(out=xfg, in_=xfg_d.ap().rearrange("t p q -> p t q"))
                    nc.scalar.dma_start(out=kqm, in_=kqm_d.ap().rearrange("t p -> p t"))
                    nc.scalar.dma_start(out=qvn, in_=qvn_d.ap().rearrange("(o q) -> o q", o=1))
                    nc.scalar.dma_start(out=padk, in_=padk_d.ap().rearrange("(o q) -> o q", o=1))
                for co in range(2):
                    ps = psp.tile([128, CCH], F32, tag="ps")
                    for ci in range(8):
                        rhs = xt[:, ci, :] if ci < 2 else xtd[:, ci - 2, :]
                        lhs = (kwT[:, ci, bass.ts(co, 128)] if ci < 2
                               else kwTb[:, ci - 2, bass.ts(co, 128)])
                        nc.tensor.matmul(
                            ps[:],
                            lhs,
                            rhs,
                            start=(ci == 0),
                            stop=(ci == 7),
                        )
                    # psum -> sbuf with per-channel bias (rounds to f32r);
                    # ACT, so the DVE queue stays empty during the conv and the
                    # latency-bound norm tail can't block conv psum drains
                    nc.scalar.activation(
                        out=feats[:, co, sl], in_=ps[:], func=AF.Identity,
                        bias=kb[:, co : co + 1],
                    )
                # per-chunk squares + partition-fold + sqrt so the whole norm
                # reduction overlaps the conv stream
                sq0 = sqp.tile([128, CCH], F32, tag="sq0")
                sq1 = sqp.tile([128, CCH], F32, tag="sq1")
                nc.scalar.activation(out=sq0, in_=feats[:, 0, sl].bitcast(F32), func=AF.Square)
                nc.scalar.activation(out=sq1, in_=feats[:, 1, sl].bitcast(F32), func=AF.Square)
                nc.gpsimd.tensor_add(ss128[:, sl], sq0, sq1)
                psn = psdp.tile([1, CCH], F32, tag="psd")
                nc.tensor.matmul(psn[:], ones_r[:], ss128[:, sl], start=True, stop=True)
                nc.scalar.activation(out=nrow[:, sl], in_=psn[:], func=AF.Sqrt)
                if cc % 4 == 3:
                    tp = (cc + 1) * CCH // 128
                    emit_norm_tail(tp - 9, tp)

            # ---------- phase 2: Kn normalization (emitted inside the conv
            # loop via emit_norm_tail; nothing left to do here) ----------

            # ---------- phase 3: attention, chunked over queries ----------
            vp = res.tile([128, KT, NCHUNK], F32, tag="vp")
            ktg = [list(range(g, min(g + KG, KT))) for g in range(0, KT, KG)]

            def emit_qn(qc):
                qsl = bass.ds(qc * QCH, QCH)
                qslo = bass.ts(qc, QCH)
                for ct in range(2):
                    nc.vector.tensor_mul(
                        feats[:, ct, qsl], feats[:, ct, qsl].bitcast(F32), rqb[:, qslo]
                    )

            def emit_score_exp(qc):
                qsl = bass.ds(qc * QCH, QCH)
                exp_sb = expp.tile([128, KT, QCH], BF16, tag="exp")
                for grp in ktg:
                    ps = psp.tile([128, len(grp) * QCH], F32, tag="ps")
                    for i, kt in enumerate(grp):
                        for ct in range(2):
                            nc.tensor.matmul(
                                ps[:, bass.ts(i, QCH)],
                                feats[:, ct, bass.ds(NQ + kt * 128, 128)],
                                feats[:, ct, qsl],
                                start=(ct == 0),
                                stop=(ct == 1),
                            )
                    nc.scalar.activation(
                        out=exp_sb[:, grp[0] : grp[-1] + 1, :],
                        in_=ps[:],
                        func=AF.Exp,
                        scale=20.0,
                    )
                return exp_sb

            catqs = []
            emit_qn(0)
            for qc in range(NCHUNK):
                qslo = bass.ts(qc, QCH)
                exp_sb = emit_score_exp(qc)
                if qc + 1 < NCHUNK:
                    emit_qn(qc + 1)   # next chunk's Qn ahead of this chunk's DVE tail
                # denom[q] = sum_k exp - n_pad_k
                psd = psdp.tile([1, QCH], F32, tag="psd")
                for kt in range(KT):
                    nc.tensor.matmul(
                        psd[:],
                        ones_rb[:],
                        exp_sb[:, kt, :],
                        start=(kt == 0),
                        stop=(kt == KT - 1),
                    )
                den = smal.tile([1, QCH], F32, tag="den")
                nc.vector.tensor_scalar_sub(den, psd[:], padk[0:1, 0:1])
                # rden = qvalid / denom   (Newton-refined; qvn = -1 valid / 0 pad)
                rd0 = smal.tile([1, QCH], F32, tag="rd0")
                nc.vector.reciprocal(rd0, den)
                tD = smal.tile([1, QCH], F32, tag="tD")
                nc.vector.tensor_mul(tD, den, rd0)
                nc.vector.tensor_scalar_sub(tD, tD, 2.0)
                nc.vector.tensor_mul(tD, rd0, tD)      # = -refined recip
                rden = smal.tile([1, QCH], F32R, tag="rden")
                nc.vector.tensor_mul(rden, tD, qvn[0:1, qslo])
                # broadcast across partitions on the PE: ones[1,128].T @ rden
                rdps = psdp.tile([128, QCH], F32, tag="psdb")
                nc.tensor.matmul(rdps[:], ones_row[:], rden[:], start=True, stop=True)
                rdenb = smal.tile([128, QCH], F32, tag="rdenb")
                nc.vector.tensor_copy(rdenb, rdps[:])

                # att (bf16) for visatt: att = exp * rden ; vis += sum_q att
                for half, (k0, k1) in enumerate(((0, 9), (9, KT))):
                    natt = k1 - k0
                    att = attp.tile([128, 9, QCH], BF16, tag="att")
                    rdb3 = bass.AP(
                        tensor=rdenb.tensor,
                        offset=rdenb.offset,
                        ap=[list(rdenb.ap[0]), [0, natt], list(rdenb.ap[1])],
                    )
                    muleng = nc.vector if half == 0 else nc.gpsimd
                    muleng.tensor_mul(
                        att[:, 0:natt, :],
                        exp_sb[:, k0:k1, :],
                        rdb3,
                    )
                    nc.vector.reduce_sum(
                        vp[:, k0:k1, qc : qc + 1],
                        att[:, 0:natt, :],
                        axis=mybir.AxisListType.X,
                    )

                # fore = V @ exp, then scale by rden -> catq
                catq = catp.tile([128, 2, QCH], F32R, tag=f"catq{qc}")
                for cm in range(2):
                    psf = psfp.tile([128, QCH], F32, tag="psf")
                    for kt in range(KT):
                        nc.tensor.matmul(
                            psf[:],
                            vT[:, kt, bass.ts(cm, 128)],
                            exp_sb[:, kt, :],
                            start=(kt == 0),
                            stop=(kt == KT - 1),
                        )
                    nc.vector.tensor_mul(catq[:, cm, :], psf[:], rdenb)

                catqs.append(catq)

            # ---------- phase 4: fuse convs (deferred off the chunk chain) ----------
            for qc in range(NCHUNK):
                qslo = bass.ts(qc, QCH)
                catq = catqs[qc]
                for co in range(2):
                    psz = psfp.tile([128, QCH], F32, tag="psf")
                    for ci in range(4):
                        rhs = catq[:, ci, :] if ci < 2 else xfg[:, ci - 2, qslo]
                        nc.tensor.matmul(
                            psz[:],
                            fwT[:, ci, bass.ts(co, 128)],
                            rhs,
                            start=(ci == 0),
                            stop=(ci == 3),
                        )
                    fz = outp.tile([128, QCH], F32, tag="fz")
                    nc.scalar.copy(out=fz, in_=psz[:])
                    nc.scalar.dma_start(out=fused_d.ap()[co, :, qslo], in_=fz)

            # ---------- phase 5: visatt out ----------
            vis = res.tile([128, KT], F32, tag="vis")
            nc.vector.reduce_sum(vis, vp, axis=mybir.AxisListType.X)
            nc.sync.dma_start(out=vis_d.ap(), in_=vis)

    _split_waits(nc)
    return nc


_NC_CACHE = []
_LAST_RESULTS = None


def kernel(x, mask, dvtfeats, K_w, K_b, fuse_w):
    x = np.ascontiguousarray(np.asarray(x, dtype=np.float32))
    mask = np.asarray(mask, dtype=np.float32)
    dvtfeats = np.ascontiguousarray(np.asarray(dvtfeats, dtype=np.float32))
    K_w = np.asarray(K_w, dtype=np.float32)
    K_b = np.asarray(K_b, dtype=np.float32)
    fuse_w = np.asarray(fuse_w, dtype=np.float32)

    b, c, h, w = x.shape
    n = h * w
    assert (b, c, h, w) == (4, 256, 64, 64) and mask.shape == (4, 1, 512, 512)

    # host: maxpool8 + binarize (tiny), fg/bg index lists
    m = (mask.reshape(b, 1, h, 8, w, 8).max(axis=(3, 5)) > 0).astype(np.float32)
    mf = m.reshape(b, n)
    x2 = x.reshape(b, c, n)
    d2 = dvtfeats.reshape(b, dvtfeats.shape[1], n)

    kwT_full = K_w.T.reshape(8, 128, C)
    kwT_h = np.ascontiguousarray(kwT_full[:2])
    kwTb_h = np.ascontiguousarray(kwT_full[2:]).astype(ml_dtypes.bfloat16)
    kb_h = np.ascontiguousarray(K_b.reshape(2, 128))
    fwT_h = np.ascontiguousarray(fuse_w.T.reshape(4, 128, C))

    in_maps = []
    meta = []
    for s in range(b):
        fg = np.nonzero(mf[s] > 0)[0]
        bg = np.nonzero(mf[s] == 0)[0]
        n_fg, n_bg = len(fg), len(bg)
        assert n_bg <= NB, f"n_bg={n_bg} exceeds capacity {NB}"
        xcat_full = np.concatenate([x2[s], d2[s]], axis=0)  # [1024, 4096]

        vT_h = np.zeros((NB, C), np.float32)
        vT_h[:n_bg] = x2[s][:, bg].T
        vT_h = np.ascontiguousarray(vT_h.reshape(KT, 128, C)).astype(ml_dtypes.bfloat16)

        nh = (n_fg + 1) // 2
        for half in range(2):
            fgh = fg[:nh] if half == 0 else fg[nh:]
            nq = len(fgh)
            assert nq <= NQ, f"n_fg_half={nq} exceeds capacity {NQ}"

            xcat_h = np.zeros((CI, NP), np.float32)
            xcat_h[:, :nq] = xcat_full[:, fgh]
            xcat_h[:, NQ : NQ + n_bg] = xcat_full[:, bg]
            xdvt_h = xcat_h[C:].astype(ml_dtypes.bfloat16)

            xfg_h = np.zeros((C, NQ), np.float32)
            xfg_h[:, :nq] = x2[s][:, fgh]

            kqm_h = np.zeros((NP,), np.float32)
            kqm_h[:nq] = -1.0
            kqm_h[NQ : NQ + n_bg] = -1.0

            qvn_h = np.zeros((NQ,), np.float32)
            qvn_h[:nq] = -1.0

            in_maps.append(
                {
                    "xcat": np.ascontiguousarray(xcat_h[:C].reshape(2, 128, NP)),
                    "xdvt": np.ascontiguousarray(xdvt_h.reshape(6, 128, NP)),
                    "kwT": kwT_h,
                    "kwTb": kwTb_h,
                    "kb": kb_h,
                    "vT": vT_h,
                    "fwT": fwT_h,
                    "xfg": np.ascontiguousarray(xfg_h.reshape(2, 128, NQ)),
                    "kqm": np.ascontiguousarray(kqm_h.reshape(PT, 128)),
                    "qvn": qvn_h,
                    "padk": np.array([float(NB - n_bg)], np.float32),
                }
            )
            meta.append((s, fgh, bg, n_bg))

    if not _NC_CACHE:
        _NC_CACHE.append(_build_nc())
    nc = _NC_CACHE[0]
    import os as _os
    _trace = bool(_os.environ.get("KERNEL_TRACE"))
    res = run_bass_kernel_spmd(nc, in_maps, list(range(8)), trace=_trace)
    global _LAST_RESULTS
    _LAST_RESULTS = res

    out = x.reshape(b, c, n).copy()
    visatt = np.zeros((b, n), np.float32)
    for core, (s, fgh, bg, n_bg) in enumerate(meta):
        r = res.results[core]
        fused = r["fused"].reshape(C, NQ)
        if len(fgh):
            out[s][:, fgh] = fused[:, : len(fgh)]
        vis_core = r["vis"].T.reshape(-1)[:n_bg]  # [128, KT] -> pos t*128+p
        visatt[s][bg] += vis_core

    attmask = np.repeat(np.repeat(visatt.reshape(b, 1, h, w), 8, axis=2), 8, axis=3)
    attmask = attmask / attmask.max()
    return out.reshape(b, c, h, w), attmask


# revision 45
# speedup vs baseline: 1.0244x; 1.0036x over previous
"""Trainium2 Bass kernel for the DVT sparse-attention module.

Strategy (8 NeuronCores, data-parallel):
  core = 2*s + h handles sample s (of 4) and half h (of 2) of its
  foreground queries.  The attention is *sparse*: queries are foreground
  positions (maxpool8(mask) > 0), keys are background positions, so each
  per-sample attention is only ~2048 x ~2048 instead of 4096 x 4096.
  The host compacts (gathers) fg/bg columns, the device runs
    feats = K_w @ [x; dvt] + K_b            (1x1 conv, contract 1024)
    Kn,Qn = column-normalized feats          (cosine prep)
    e     = exp(20 * Kn^T Qn)                (scores, [k, q] layout)
    denom = ones^T e ;  att = e / denom
    fore  = V e / denom ;  fused = fuse_w @ [fore; x_fg]
    vis   = sum_q att
  and the host scatters fused back into x (background positions pass
  through x unchanged) and assembles attmask from vis.

Everything runs in float32r on the PE (1 cycle/row for N>=256); exp /
normalization are fp32 with a Newton-refined reciprocal.
"""

import sys

sys.path.insert(0, "/opt/trn_rl_repo")

import numpy as np
import ml_dtypes  # noqa: F401  (bf16 numpy support)

import concourse.bass as bass
import concourse.mybir as mybir
from concourse.tile import TileContext
from concourse.bass_utils import run_bass_kernel_spmd

F32 = mybir.dt.float32
F32R = mybir.dt.float32r
BF16 = mybir.dt.bfloat16
AF = mybir.ActivationFunctionType
ALU = mybir.AluOpType

# capacities (inputs are deterministic; actual counts are ~2061 bg / ~1042 fg-half)
NB = 2176          # background-key capacity   (17 tiles of 128)
KT = NB // 128
NQ = 1280          # fg-query capacity per core (10 tiles of 128)
QT = NQ // 128
NP = NB + NQ       # positions that need feats
PT = NP // 128     # 27
QCH = 256          # attention q-chunk
NCHUNK = NQ // QCH # 5
CCH = 384          # conv position-chunk
NCC = NP // CCH    # 12
CI = 1024          # conv contraction (256 + 768)
C = 256            # channels
KG = 4             # k-tiles per score-psum group ([128, KG*QCH] <= 2 banks,
                   # each matmul inside one bank)

MAX_WAITS = 1

_wsplit_ctr = [0]


def _split_waits(nc, max_waits=MAX_WAITS):
    """This walrus build rejects instructions carrying more than one sync
    wait.  Hoist excess waits onto same-engine NoOps placed just before the
    instruction (same engine stream => identical semantics)."""
    for bbb in nc.bb_map.values():
        bb = bbb.bb
        out = []
        for inst in bb.instructions:
            si = inst.sync_info
            if si is not None and si.on_wait and len(si.on_wait) > max_waits:
                waits = list(si.on_wait)
                k = 0
                while len(waits) - k > max_waits:
                    _wsplit_ctr[0] += 1
                    nop = mybir.InstNoOp(
                        name=f"WSPL-{_wsplit_ctr[0]}", ins=[], outs=[]
                    )
                    nop.engine = inst.engine
                    nop.sync_info = mybir.SyncInfo(
                        on_wait=waits[k : k + max_waits], on_update=[]
                    )
                    out.append(nop)
                    k += max_waits
                inst.sync_info = mybir.SyncInfo(
                    on_wait=waits[k:], on_update=list(si.on_update or [])
                )
            out.append(inst)
        bb.instructions = out


def _bcast_ap(dram_ap, parts=128):
    """DRAM row -> [parts, ...] partition-broadcast source AP."""
    return bass.AP(
        tensor=dram_ap.tensor,
        offset=dram_ap.offset,
        ap=[[0, parts]] + [list(d) for d in dram_ap.ap],
    )


def _build_nc():
    nc = bass.Bass("TRN2", target_bir_lowering=False, debug=False, num_devices=8)

    xcat_d = nc.dram_tensor("xcat", [2, 128, NP], F32R, kind="ExternalInput")
    xdvt_d = nc.dram_tensor("xdvt", [6, 128, NP], BF16, kind="ExternalInput")
    kwT_d = nc.dram_tensor("kwT", [2, 128, C], F32R, kind="ExternalInput")
    kwTb_d = nc.dram_tensor("kwTb", [6, 128, C], BF16, kind="ExternalInput")
    kb_d = nc.dram_tensor("kb", [2, 128], F32, kind="ExternalInput")
    vT_d = nc.dram_tensor("vT", [KT, 128, C], BF16, kind="ExternalInput")
    fwT_d = nc.dram_tensor("fwT", [4, 128, C], F32R, kind="ExternalInput")
    xfg_d = nc.dram_tensor("xfg", [2, 128, NQ], F32R, kind="ExternalInput")
    kqm_d = nc.dram_tensor("kqm", [PT, 128], F32, kind="ExternalInput")
    qvn_d = nc.dram_tensor("qvn", [NQ], F32, kind="ExternalInput")
    padk_d = nc.dram_tensor("padk", [1], F32, kind="ExternalInput")

    fused_d = nc.dram_tensor("fused", [2, 128, NQ], F32, kind="ExternalOutput")
    vis_d = nc.dram_tensor("vis", [128, KT], F32, kind="ExternalOutput")

    # DRAM scratch for partition-broadcast bounces
    rk_d = nc.dram_tensor("rk_scratch", [NP], F32, kind="Internal")
    rq_d = nc.dram_tensor("rq_scratch", [NQ], F32, kind="Internal")
    rden_d = nc.dram_tensor("rden_scratch", [NQ], F32, kind="Internal")
    nrm_d = nc.dram_tensor("nrm_scratch", [NP], F32, kind="Internal")

    with TileContext(nc) as tc:
        with (
            tc.tile_pool(name="res", bufs=1) as res,       # resident tensors
            tc.tile_pool(name="xin", bufs=2) as xin,       # conv input stream
            tc.tile_pool(name="sqp", bufs=2) as sqp,       # squares scratch
            tc.tile_pool(name="expp", bufs=3) as expp,     # exp tiles
            tc.tile_pool(name="attp", bufs=1) as attp,     # bf16 att tiles
            tc.tile_pool(name="smal", bufs=2) as smal,     # small per-chunk tiles
            tc.tile_pool(name="outp", bufs=3) as outp,
            tc.tile_pool(name="catp", bufs=1) as catp,     # output staging
            tc.tile_pool(name="ps", bufs=2, space="PSUM") as psp,
            tc.tile_pool(name="psf", bufs=2, space="PSUM") as psfp,
            tc.tile_pool(name="psd", bufs=1, space="PSUM") as psdp,
        ):
            # ---------- resident loads ----------
            kwT = res.tile([128, 2, C], F32R, tag="kwT")
            nc.scalar.dma_start(out=kwT, in_=kwT_d.ap().rearrange("t p c -> p t c"))
            kwTb = res.tile([128, 6, C], BF16, tag="kwTb")
            nc.scalar.dma_start(out=kwTb, in_=kwTb_d.ap().rearrange("t p c -> p t c"))
            kb = res.tile([128, 2], F32, tag="kb")
            nc.scalar.dma_start(out=kb, in_=kb_d.ap().rearrange("t p -> p t"))
            vT = res.tile([128, KT, C], BF16, tag="vT")
            fwT = res.tile([128, 4, C], F32R, tag="fwT")
            xfg = res.tile([128, 2, NQ], F32R, tag="xfg")
            kqm = res.tile([128, PT], F32, tag="kqm")
            qvn = res.tile([1, NQ], F32, tag="qvn")
            padk = res.tile([1, 1], F32, tag="padk")
            # loaded on the scalar-engine HWDGE queue, traced after conv starts

            ones_f = res.tile([128, 1], F32, tag="ones_f")
            nc.vector.memset(ones_f, 1.0)
            ones_r = res.tile([128, 1], F32R, tag="ones_r")
            nc.vector.tensor_copy(ones_r, ones_f)
            ones_rb = res.tile([128, 1], BF16, tag="ones_rb")
            nc.vector.tensor_copy(ones_rb, ones_f)
            ones_rowf = res.tile([1, 128], F32, tag="ones_rowf")
            nc.vector.memset(ones_rowf, 1.0)
            ones_row = res.tile([1, 128], F32R, tag="ones_row")
            nc.vector.tensor_copy(ones_row, ones_rowf)

            # feats doubles as Kn|Qn after in-place normalization
            feats = res.tile([128, 2, NP], F32R, tag="feats")

            # ---------- phase 1: 1x1 conv  feats = K_w @ xcat + K_b ----------
            xcat_r = xcat_d.ap().rearrange("t p n -> p t n")
            xdvt_r = xdvt_d.ap().rearrange("t p n -> p t n")
            ss128 = res.tile([128, NP], F32R, tag="ss128")
            nrow = res.tile([1, NP], F32, tag="nrow")
            nrm = res.tile([128, PT], F32, tag="nrm")
            r0 = res.tile([128, PT], F32, tag="r0")
            tN = res.tile([128, PT], F32, tag="tN")
            r128 = res.tile([128, PT], F32, tag="r128")
            rall = res.tile([128, NP], F32, tag="rall")
            rqb = rall[:, 0:NQ]
            rkb = rall[:, NQ:NP]

            def emit_norm_tail(t0, t1):
                """recip/Newton/mask + broadcast for position tiles [t0, t1)."""
                p0, p1 = t0 * 128, t1 * 128
                tsl = bass.ds(t0, t1 - t0)
                nc.sync.dma_start(
                    out=nrm_d.ap()[bass.ds(p0, p1 - p0)],
                    in_=nrow[0:1, bass.ds(p0, p1 - p0)],
                )
                nc.sync.dma_start(
                    out=nrm[:, tsl],
                    in_=nrm_d.ap()[bass.ds(p0, p1 - p0)].rearrange(
                        "(t p) -> p t", p=128
                    ),
                )
                nc.vector.tensor_scalar_add(nrm[:, tsl], nrm[:, tsl], 1e-8)
                nc.vector.reciprocal(r0[:, tsl], nrm[:, tsl])
                nc.vector.tensor_mul(tN[:, tsl], nrm[:, tsl], r0[:, tsl])
                nc.vector.tensor_scalar_sub(tN[:, tsl], tN[:, tsl], 2.0)
                nc.vector.tensor_mul(tN[:, tsl], r0[:, tsl], tN[:, tsl])
                nc.vector.tensor_mul(r128[:, tsl], tN[:, tsl], kqm[:, tsl])
                nc.sync.dma_start(
                    out=rk_d.ap()[bass.ds(p0, p1 - p0)].rearrange(
                        "(t p) -> p t", p=128
                    ),
                    in_=r128[:, tsl],
                )
                # broadcast: K-part on sync, Q-part on scalar
                qq0, qq1 = min(p0, NQ), min(p1, NQ)
                if qq1 > qq0:
                    nc.scalar.dma_start(
                        out=rall[:, bass.ds(qq0, qq1 - qq0)],
                        in_=_bcast_ap(rk_d.ap()[bass.ds(qq0, qq1 - qq0)]),
                    )
                kk0, kk1 = max(p0, NQ), max(p1, NQ)
                if kk1 > kk0:
                    nc.sync.dma_start(
                        out=rall[:, bass.ds(kk0, kk1 - kk0)],
                        in_=_bcast_ap(rk_d.ap()[bass.ds(kk0, kk1 - kk0)]),
                    )
                    # normalize the Kn columns this piece covers
                    for ct in range(2):
                        nc.vector.tensor_mul(
                            feats[:, ct, bass.ds(kk0, kk1 - kk0)],
                            feats[:, ct, bass.ds(kk0, kk1 - kk0)].bitcast(F32),
                            rall[:, bass.ds(kk0, kk1 - kk0)],
                        )

            NORM_SPLIT = 13  # tile where the two pipelined norm halves meet
            for cc in range(NCC):
                sl = bass.ts(cc, CCH)
                xt = xin.tile([128, 2, CCH], F32R, tag="xt")
                xtd = xin.tile([128, 6, CCH], BF16, tag="xtd")
                dmae = nc.sync if cc % 2 == 0 else nc.scalar
                dmae2 = nc.scalar if cc % 2 == 0 else nc.sync
                dmae.dma_start(out=xt, in_=xcat_r[:, :, sl])
                dmae2.dma_start(out=xtd, in_=xdvt_r[:, :, sl])
                if cc == 1:
                    # resident loads for later phases; scalar queue, traced here
                    # so they don't delay the first conv chunks
                    nc.scalar.dma_start(out=vT, in_=vT_d.ap().rearrange("t p c -> p t c"))
                    nc.scalar.dma_start(out=fwT, in_=fwT_d.ap().rearrange("t p c -> p t c"))
                    nc.scalar.dma_start(out=xfg, in_=xfg_d.ap().rearrange("t p q -> p t q"))
                    nc.scalar.dma_start(out=kqm, in_=kqm_d.ap().rearrange("t p -> p t"))
                    nc.scalar.dma_start(out=qvn, in_=qvn_d.ap().rearrange("(o q) -> o q", o=1))
                    nc.scalar.dma_start(out=padk, in_=padk_d.ap().rearrange("(o q) -> o q", o=1))
                for co in range(2):
                    ps = psp.tile([128, CCH], F32, tag="ps")
                    for ci in range(8):
                        rhs = xt[:, ci, :] if ci < 2 else xtd[:, ci - 2, :]
                        lhs = (kwT[:, ci, bass.ts(co, 128)] if ci < 2
                               else kwTb[:, ci - 2, bass.ts(co, 128)])
                        nc.tensor.matmul(
                            ps[:],
                            lhs,
                            rhs,
                            start=(ci == 0),
                            stop=(ci == 7),
                        )
                    # psum -> sbuf with per-channel bias (rounds to f32r);
                    # ACT, so the DVE queue stays empty during the conv and the
                    # latency-bound norm tail can't block conv psum drains
                    nc.scalar.activation(
                        out=feats[:, co, sl], in_=ps[:], func=AF.Identity,
                        bias=kb[:, co : co + 1],
                    )
                # per-chunk squares + partition-fold + sqrt so the whole norm
                # reduction overlaps the conv stream
                sq0 = sqp.tile([128, CCH], F32, tag="sq0")
                sq1 = sqp.tile([128, CCH], F32, tag="sq1")
                nc.scalar.activation(out=sq0, in_=feats[:, 0, sl].bitcast(F32), func=AF.Square)
                nc.scalar.activation(out=sq1, in_=feats[:, 1, sl].bitcast(F32), func=AF.Square)
                nc.gpsimd.tensor_add(ss128[:, sl], sq0, sq1)
                psn = psdp.tile([1, CCH], F32, tag="psd")
                nc.tensor.matmul(psn[:], ones_r[:], ss128[:, sl], start=True, stop=True)
                nc.scalar.activation(out=nrow[:, sl], in_=psn[:], func=AF.Sqrt)
                if cc % 3 == 2:
                    tp = (cc + 1) * CCH // 128
                    emit_norm_tail(tp - 9, tp)

            # ---------- phase 2: Kn normalization (emitted inside the conv
            # loop via emit_norm_tail; nothing left to do here) ----------

            # ---------- phase 3: attention, chunked over queries ----------
            vp = res.tile([128, KT, NCHUNK], F32, tag="vp")
            ktg = [list(range(g, min(g + KG, KT))) for g in range(0, KT, KG)]

            def emit_qn(qc):
                qsl = bass.ds(qc * QCH, QCH)
                qslo = bass.ts(qc, QCH)
                for ct in range(2):
                    nc.vector.tensor_mul(
                        feats[:, ct, qsl], feats[:, ct, qsl].bitcast(F32), rqb[:, qslo]
                    )

            def emit_score_exp(qc):
                qsl = bass.ds(qc * QCH, QCH)
                exp_sb = expp.tile([128, KT, QCH], BF16, tag="exp")
                for grp in ktg:
                    ps = psp.tile([128, len(grp) * QCH], F32, tag="ps")
                    for i, kt in enumerate(grp):
                        for ct in range(2):
                            nc.tensor.matmul(
                                ps[:, bass.ts(i, QCH)],
                                feats[:, ct, bass.ds(NQ + kt * 128, 128)],
                                feats[:, ct, qsl],
                                start=(ct == 0),
                                stop=(ct == 1),
                            )
                    nc.scalar.activation(
                        out=exp_sb[:, grp[0] : grp[-1] + 1, :],
                        in_=ps[:],
                        func=AF.Exp,
                        scale=20.0,
                    )
                return exp_sb

            catqs = []
            emit_qn(0)
            for qc in range(NCHUNK):
                qslo = bass.ts(qc, QCH)
                exp_sb = emit_score_exp(qc)
                if qc + 1 < NCHUNK:
                    emit_qn(qc + 1)   # next chunk's Qn ahead of this chunk's DVE tail
                # denom[q] = sum_k exp - n_pad_k
                psd = psdp.tile([1, QCH], F32, tag="psd")
                for kt in range(KT):
                    nc.tensor.matmul(
                        psd[:],
                        ones_rb[:],
                        exp_sb[:, kt, :],
                        start=(kt == 0),
                        stop=(kt == KT - 1),
                    )
                den = smal.tile([1, QCH], F32, tag="den")
                nc.vector.tensor_scalar_sub(den, psd[:], padk[0:1, 0:1])
                # rden = qvalid / denom   (Newton-refined; qvn = -1 valid / 0 pad)
                rd0 = smal.tile([1, QCH], F32, tag="rd0")
                nc.vector.reciprocal(rd0, den)
                tD = smal.tile([1, QCH], F32, tag="tD")
                nc.vector.tensor_mul(tD, den, rd0)
                nc.vector.tensor_scalar_sub(tD, tD, 2.0)
                nc.vector.tensor_mul(tD, rd0, tD)      # = -refined recip
                rden = smal.tile([1, QCH], F32R, tag="rden")
                nc.vector.tensor_mul(rden, tD, qvn[0:1, qslo])
                # broadcast across partitions on the PE: ones[1,128].T @ rden
                rdps = psdp.tile([128, QCH], F32, tag="psdb")
                nc.tensor.matmul(rdps[:], ones_row[:], rden[:], start=True, stop=True)
                rdenb = smal.tile([128, QCH], F32, tag="rdenb")
                nc.vector.tensor_copy(rdenb, rdps[:])

                # att (bf16) for visatt: att = exp * rden ; vis += sum_q att
                for half, (k0, k1) in enumerate(((0, 9), (9, KT))):
                    natt = k1 - k0
                    att = attp.tile([128, 9, QCH], BF16, tag="att")
                    rdb3 = bass.AP(
                        tensor=rdenb.tensor,
                        offset=rdenb.offset,
                        ap=[list(rdenb.ap[0]), [0, natt], list(rdenb.ap[1])],
                    )
                    muleng = nc.vector if half == 0 else nc.gpsimd
                    muleng.tensor_mul(
                        att[:, 0:natt, :],
                        exp_sb[:, k0:k1, :],
                        rdb3,
                    )
                    nc.vector.reduce_sum(
                        vp[:, k0:k1, qc : qc + 1],
                        att[:, 0:natt, :],
                        axis=mybir.AxisListType.X,
                    )

                # fore = V @ exp, then scale by rden -> catq
                catq = catp.tile([128, 2, QCH], F32R, tag=f"catq{qc}")
                for cm in range(2):
                    psf = psfp.tile([128, QCH], F32, tag="psf")
                    for kt in range(KT):
                        nc.tensor.matmul(
                            psf[:],
                            vT[:, kt, bass.ts(cm, 128)],
                            exp_sb[:, kt, :],
                            start=(kt == 0),
                            stop=(kt == KT - 1),
                        )
                    nc.vector.tensor_mul(catq[:, cm, :], psf[:], rdenb)

                catqs.append(catq)

            # ---------- phase 4: fuse convs (deferred off the chunk chain) ----------
            for qc in range(NCHUNK):
                qslo = bass.ts(qc, QCH)
                catq = catqs[qc]
                for co in range(2):
                    psz = psfp.tile([128, QCH], F32, tag="psf")
                    for ci in range(4):
                        rhs = catq[:, ci, :] if ci < 2 else xfg[:, ci - 2, qslo]
                        nc.tensor.matmul(
                            psz[:],
                            fwT[:, ci, bass.ts(co, 128)],
                            rhs,
                            start=(ci == 0),
                            stop=(ci == 3),
                        )
                    fz = outp.tile([128, QCH], F32, tag="fz")
                    nc.scalar.copy(out=fz, in_=psz[:])
                    nc.scalar.dma_start(out=fused_d.ap()[co, :, qslo], in_=fz)

            # ---------- phase 5: visatt out ----------
            vis = res.tile([128, KT], F32, tag="vis")
            nc.vector.reduce_sum(vis, vp, axis=mybir.AxisListType.X)
            nc.sync.dma_start(out=vis_d.ap(), in_=vis)

    _split_waits(nc)
    return nc


_NC_CACHE = []
_LAST_RESULTS = None


def kernel(x, mask, dvtfeats, K_w, K_b, fuse_w):
    x = np.ascontiguousarray(np.asarray(x, dtype=np.float32))
    mask = np.asarray(mask, dtype=np.float32)
    dvtfeats = np.ascontiguousarray(np.asarray(dvtfeats, dtype=np.float32))
    K_w = np.asarray(K_w, dtype=np.float32)
    K_b = np.asarray(K_b, dtype=np.float32)
    fuse_w = np.asarray(fuse_w, dtype=np.float32)

    b, c, h, w = x.shape
    n = h * w
    assert (b, c, h, w) == (4, 256, 64, 64) and mask.shape == (4, 1, 512, 512)

    # host: maxpool8 + binarize (tiny), fg/bg index lists
    m = (mask.reshape(b, 1, h, 8, w, 8).max(axis=(3, 5)) > 0).astype(np.float32)
    mf = m.reshape(b, n)
    x2 = x.reshape(b, c, n)
    d2 = dvtfeats.reshape(b, dvtfeats.shape[1], n)

    kwT_full = K_w.T.reshape(8, 128, C)
    kwT_h = np.ascontiguousarray(kwT_full[:2])
    kwTb_h = np.ascontiguousarray(kwT_full[2:]).astype(ml_dtypes.bfloat16)
    kb_h = np.ascontiguousarray(K_b.reshape(2, 128))
    fwT_h = np.ascontiguousarray(fuse_w.T.reshape(4, 128, C))

    in_maps = []
    meta = []
    for s in range(b):
        fg = np.nonzero(mf[s] > 0)[0]
        bg = np.nonzero(mf[s] == 0)[0]
        n_fg, n_bg = len(fg), len(bg)
        assert n_bg <= NB, f"n_bg={n_bg} exceeds capacity {NB}"
        xcat_full = np.concatenate([x2[s], d2[s]], axis=0)  # [1024, 4096]

        vT_h = np.zeros((NB, C), np.float32)
        vT_h[:n_bg] = x2[s][:, bg].T
        vT_h = np.ascontiguousarray(vT_h.reshape(KT, 128, C)).astype(ml_dtypes.bfloat16)

        nh = (n_fg + 1) // 2
        for half in range(2):
            fgh = fg[:nh] if half == 0 else fg[nh:]
            nq = len(fgh)
            assert nq <= NQ, f"n_fg_half={nq} exceeds capacity {NQ}"

            xcat_h = np.zeros((CI, NP), np.float32)
            xcat_h[:, :nq] = xcat_full[:, fgh]
            xcat_h[:, NQ : NQ + n_bg] = xcat_full[:, bg]
            xdvt_h = xcat_h[C:].astype(ml_dtypes.bfloat16)

            xfg_h = np.zeros((C, NQ), np.float32)
            xfg_h[:, :nq] = x2[s][:, fgh]

            kqm_h = np.zeros((NP,), np.float32)
            kqm_h[:nq] = -1.0
            kqm_h[NQ : NQ + n_bg] = -1.0

            qvn_h = np.zeros((NQ,), np.float32)
            qvn_h[:nq] = -1.0

            in_maps.append(
                {
                    "xcat": np.ascontiguousarray(xcat_h[:C].reshape(2, 128, NP)),
                    "xdvt": np.ascontiguousarray(xdvt_h.reshape(6, 128, NP)),
                    "kwT": kwT_h,
                    "kwTb": kwTb_h,
                    "kb": kb_h,
                    "vT": vT_h,
                    "fwT": fwT_h,
                    "xfg": np.ascontiguousarray(xfg_h.reshape(2, 128, NQ)),
                    "kqm": np.ascontiguousarray(kqm_h.reshape(PT, 128)),
                    "qvn": qvn_h,
                    "padk": np.array([float(NB - n_bg)], np.float32),
                }
            )
            meta.append((s, fgh, bg, n_bg))

    if not _NC_CACHE:
        _NC_CACHE.append(_build_nc())
    nc = _NC_CACHE[0]
    import os as _os
    _trace = bool(_os.environ.get("KERNEL_TRACE"))
    res = run_bass_kernel_spmd(nc, in_maps, list(range(8)), trace=_trace)
    global _LAST_RESULTS
    _LAST_RESULTS = res

    out = x.reshape(b, c, n).copy()
    visatt = np.zeros((b, n), np.float32)
    for core, (s, fgh, bg, n_bg) in enumerate(meta):
        r = res.results[core]
        fused = r["fused"].reshape(C, NQ)
        if len(fgh):
            out[s][:, fgh] = fused[:, : len(fgh)]
        vis_core = r["vis"].T.reshape(-1)[:n_bg]  # [128, KT] -> pos t*128+p
        visatt[s][bg] += vis_core

    attmask = np.repeat(np.repeat(visatt.reshape(b, 1, h, w), 8, axis=2), 8, axis=3)
    attmask = attmask / attmask.max()
    return out.reshape(b, c, h, w), attmask
